# revision 1
# baseline (speedup 1.0000x reference)
"""Trainium2 Bass kernel for CrossAttentionBlock (GN -> qkv proj -> full attention -> conv3x3; fp32 residual on host).

Sharding: 8 cores = 4 samples x 2 query-row-halves. Each core gets the full
sample's kv (all keys) and computes attention for 34 query rows (32 output rows
+ 1 halo row each side, zero-padded at image edges), then conv3x3 for its 32
rows. GroupNorm stats are computed redundantly per core from the full sample.

All heavy matmuls run in fp8e4m3 with DoubleRow (2 fp8 MACs/cell/cycle).
Scaling scheme (fp8 value ranges kept near ~0.5):
  - wq/wk/wv are pre-scaled x16 on host (raw std ~0.028 would be subnormal in
    fp8); the ACT copies out of PSUM descale by 1/16.
  - the attention 1/sqrt(C) lives in the Exp activation's scale argument.
  - v path keeps the x16 (vpT = 16*vp); rowmask carries 4.0 = 64/16 so the
    softmax-normalize produces a_pad = 64*a (healthy fp8 range).
  - wo is pre-scaled x2^22 (raw std ~2e-7); conv output ACT descales by
    2^-28 = 1/(2^22 * 64).
The final output is the tiny conv delta (~1e-5) in bf16; the fp32 residual
(+q) is added on the host, so fp8 noise lands ~1e-7 relative to output scale.
"""

import sys

if "/opt/trn_rl_repo" not in sys.path:
    sys.path.insert(0, "/opt/trn_rl_repo")

import ml_dtypes
import numpy as np

B, C, H, W = 4, 256, 64, 64
HW = H * W              # 4096
CT = C // 128           # 2 channel partition-tiles
KT = HW // 128          # 32 key tiles
GPT = 16                # groups per channel-tile (32 groups of 8 channels)
EPS = 1e-5
NROWS = 34              # 32 output rows + halo row each side
NQ = NROWS * W          # 2176 queries per core
NOUT = 32 * W           # 2048 outputs per core
CHUNKS = [(0, 512), (512, 512), (1024, 512), (1536, 512), (2048, 128)]
BF16 = ml_dtypes.bfloat16
F8 = ml_dtypes.float8_e4m3
WS = 16.0               # host pre-scale on wq/wk/wv
OS = float(2 ** 22)     # host pre-scale on wo
AS = 64.0               # a_pad carries 64*a
SC = 1.0 / 16.0         # attention 1/sqrt(C), applied inside Exp
EXP_A = (2.0 ** 23) / float(np.log(2.0)) * SC   # Schraudolph exp slope
EXP_B = float(127 * 2 ** 23 - 486411)           # Schraudolph exp bias

_CACHE = {}


def _build():
    import concourse.bass as bass
    import concourse.tile as tile
    from concourse import bacc, mybir

    f32 = mybir.dt.float32
    bf16 = mybir.dt.bfloat16
    f8 = mybir.dt.float8e4
    AF = mybir.ActivationFunctionType
    DR = mybir.MatmulPerfMode.DoubleRow

    nc = bacc.Bacc("TRN2", target_bir_lowering=False)

    q_full = nc.dram_tensor("q_full", [C, HW], bf16, kind="ExternalInput")
    kv_full = nc.dram_tensor("kv_full", [C, HW], bf16, kind="ExternalInput")
    q34 = nc.dram_tensor("q34", [C, NQ], bf16, kind="ExternalInput")
    rowmask = nc.dram_tensor("rowmask", [1, NQ], f32, kind="ExternalInput")
    # packed per-channel columns: gn_w, gn_b, bq, bo.  (bk is a per-query
    # logit offset -> softmax no-op, dropped; bv is linear -> added on host.)
    cols_d = nc.dram_tensor("cols", [C, 4], f32, kind="ExternalInput")
    # packed fp8 weights, DoubleRow layout [p, j, cols]; c_in = p + 128*j.
    # column blocks: [wqT | wkT | wvT | woT(dy,dx) x 9] each C wide
    wpack_d = nc.dram_tensor("wpack", [128, 2, 12 * C], f8, kind="ExternalInput")
    gmask_d = nc.dram_tensor("gmask", [128, GPT], f32, kind="ExternalInput")
    bmask_d = nc.dram_tensor("bmask", [GPT, 128], f32, kind="ExternalInput")
    out_half = nc.dram_tensor("out_half", [C, NOUT], bf16, kind="ExternalOutput")

    with tile.TileContext(nc) as tc, \
         tc.tile_pool(name="const", bufs=1) as constp, \
         tc.tile_pool(name="acts", bufs=1) as acts, \
         tc.tile_pool(name="cols", bufs=1) as colsp:

        # ---------------- input DMAs (sync queue order = priority) ----------
        kvt_tiles, qt_tiles = [], []
        for src, tiles, nm in ((kv_full, kvt_tiles, "kvt"), (q_full, qt_tiles, "qt")):
            for ct in range(CT):
                sl = slice(ct * 128, (ct + 1) * 128)
                xt = constp.tile([128, HW], bf16, tag=f"{nm}{ct}", name=f"{nm}{ct}")
                for d in range(2):
                    nc.sync.dma_start(xt[:, d * 2048:(d + 1) * 2048],
                                      src[sl, d * 2048:(d + 1) * 2048])
                tiles.append(xt)
        gmask_sb = constp.tile([128, GPT], f32, tag="gmask", name="gmask_sb")
        nc.sync.dma_start(gmask_sb, gmask_d[:, :])
        bmask_sb = constp.tile([GPT, 128], f32, tag="bmask", name="bmask_sb")
        nc.sync.dma_start(bmask_sb, bmask_d[:, :])
        cols_sb = []
        for ct in range(CT):
            t = constp.tile([128, 4], f32, tag=f"cols{ct}", name=f"cols{ct}")
            nc.sync.dma_start(t, cols_d[ct * 128:(ct + 1) * 128, :])
            cols_sb.append(t)
        gnw_sb = [cols_sb[ct][:, 0:1] for ct in range(CT)]
        gnb_sb = [cols_sb[ct][:, 1:2] for ct in range(CT)]
        bq_sb = [cols_sb[ct][:, 2:3] for ct in range(CT)]
        bo_sb = [cols_sb[ct][:, 3:4] for ct in range(CT)]
        wpack_sb = constp.tile([128, 2, 12 * C], f8, tag="wpack", name="wpack_sb")
        nc.sync.dma_start(wpack_sb, wpack_d[:, :, :])

        def blk(i):
            return wpack_sb[:, :, i * C:(i + 1) * C]

        wq8, wk8, wv8 = blk(0), blk(1), blk(2)
        wo8 = {(dy, dx): blk(3 + dy * 3 + dx) for dy in range(3) for dx in range(3)}

        rowmask_sb = constp.tile([1, NQ], f32, tag="rowmask", name="rowmask_sb")
        nc.gpsimd.dma_start(rowmask_sb, rowmask[:, :])
        # [128, 2, 16] so the DoubleRow pair-step is 16 B (s3_lw_dual_fp8)
        ones8 = constp.tile([128, 2, 16], f8, tag="ones8", name="ones8")
        nc.vector.memset(ones8, 1.0)
        eps16 = constp.tile([GPT, 1], f32, tag="eps16", name="eps16")
        nc.vector.memset(eps16, EPS)

        # ---------------- persistent activations (fp8 DoubleRow layouts) ----
        kvn8 = acts.tile([128, 2, HW], f8, tag="kvn8", name="kvn8")
        qn8 = acts.tile([128, 2, NQ], f8, tag="qn8", name="qn8")
        q34t = [acts.tile([128, NQ], bf16, tag=f"q34t{ct}", name=f"q34t{ct}")
                for ct in range(CT)]
        kp8 = acts.tile([128, 2, HW], f8, tag="kp8", name="kp8")
        vpT_all = acts.tile([128, KT, C], f8, tag="vpT", name="vpT_all")
        a_pad8 = acts.tile([128, 2, NROWS, W + 2], f8, tag="a_pad", name="a_pad8")
        nc.gpsimd.memset(a_pad8, 0.0)

        # ---------------- GroupNorm ----------------
        with tc.tile_pool(name="stat", bufs=4) as statp, \
             tc.tile_pool(name="gn_ps", bufs=2, space="PSUM") as gn_ps:

            def gn_cols(xt, ct, nm):
                """Per-channel (scale, bias) columns from stats of xt [128, HW]."""
                stats = statp.tile([128, 8, 6], f32, tag="stats", name=f"st_{nm}{ct}")
                for s in range(8):
                    nc.vector.bn_stats(stats[:, s, :], xt[:, s * 512:(s + 1) * 512])
                mv = statp.tile([128, 2], f32, tag="mv", name=f"mv_{nm}{ct}")
                nc.vector.bn_aggr(mv, stats)
                # mv -> (mean, E[x^2]) per channel
                sq = statp.tile([128, 1], f32, tag="sq", name=f"sq_{nm}{ct}")
                nc.vector.tensor_mul(sq, mv[:, 0:1], mv[:, 0:1])
                nc.vector.tensor_add(mv[:, 1:2], mv[:, 1:2], sq)
                gs = gn_ps.tile([GPT, 2], f32, tag="gs", name=f"gs_{nm}{ct}")
                nc.tensor.matmul(gs, gmask_sb, mv, start=True, stop=True)
                gvals = statp.tile([GPT, 2], f32, tag="gvals", name=f"gv_{nm}{ct}")
                nc.vector.tensor_copy(gvals[:, 0:1], gs[:, 0:1])
                gsq = statp.tile([GPT, 1], f32, tag="gsq", name=f"gsq_{nm}{ct}")
                nc.vector.tensor_mul(gsq, gvals[:, 0:1], gvals[:, 0:1])
                gvar = statp.tile([GPT, 1], f32, tag="gvar", name=f"gvar_{nm}{ct}")
                nc.vector.tensor_sub(gvar, gs[:, 1:2], gsq)
                gstd = statp.tile([GPT, 1], f32, tag="gstd", name=f"gstd_{nm}{ct}")
                nc.scalar.activation(gstd, gvar, AF.Sqrt, bias=eps16, scale=1.0)
                nc.vector.reciprocal(gvals[:, 1:2], gstd)
                bk_ps = gn_ps.tile([128, 2], f32, tag="bk_ps", name=f"bkps_{nm}{ct}")
                nc.tensor.matmul(bk_ps, bmask_sb, gvals, start=True, stop=True)
                scol = colsp.tile([128, 1], f32, tag=f"scol_{nm}{ct}",
                                  name=f"scol_{nm}{ct}")
                bcol = colsp.tile([128, 1], f32, tag=f"bcol_{nm}{ct}",
                                  name=f"bcol_{nm}{ct}")
                nc.vector.tensor_mul(scol, bk_ps[:, 1:2], gnw_sb[ct])
                tmpc = statp.tile([128, 1], f32, tag="tmpc", name=f"tmpc_{nm}{ct}")
                nc.vector.tensor_mul(tmpc, bk_ps[:, 0:1], scol)
                nc.vector.tensor_sub(bcol, gnb_sb[ct], tmpc)
                return scol, bcol

            for ct in range(CT):
                scol, bcol = gn_cols(kvt_tiles[ct], ct, "kv")
                nc.scalar.activation(kvn8[:, ct, :], kvt_tiles[ct], AF.Identity,
                                     bias=bcol, scale=scol)
            for ct in range(CT):
                sl = slice(ct * 128, (ct + 1) * 128)
                scol, bcol = gn_cols(qt_tiles[ct], ct, "q")
                nc.gpsimd.dma_start(q34t[ct], q34[sl, :])
                nc.scalar.activation(qn8[:, ct, :], q34t[ct], AF.Identity,
                                     bias=bcol, scale=scol)

        # ---------------- projections + attention ----------------
        # One PSUM budget for both phases (D 1 + lt 3x2 + a 1 = 8 banks).
        # Proj psum tiles ride the lt-slot rotation, emitted inside chunk 0's
        # kt loop right before the lt that consumes them, so attention starts
        # immediately and the proj copies drain on DVE behind the exp stream.
        with tc.tile_pool(name="d_ps", bufs=1, space="PSUM") as dps, \
             tc.tile_pool(name="att_lt", bufs=3, space="PSUM") as lps, \
             tc.tile_pool(name="acc_ps", bufs=1, space="PSUM") as cps, \
             tc.tile_pool(name="attsb", bufs=3) as attsb, \
             tc.tile_pool(name="wTp", bufs=34) as wTp, \
             tc.tile_pool(name="bcast", bufs=2) as bcp, \
             tc.tile_pool(name="outp", bufs=3) as outp:

            def emit_proj_block(nk):
                for ht in (4 * nk, 4 * nk + 1, 4 * nk + 2, 4 * nk + 3):
                    ps = lps.tile([128, C], f32, tag="lt_ps", name=f"vpps{ht}")
                    nc.tensor.matmul(ps, kvn8[:, :, ht * 128:(ht + 1) * 128], wv8,
                                     start=True, stop=True, perf_mode=DR)
                    nc.vector.tensor_copy(vpT_all[:, ht, :], ps)
                for ct in range(CT):
                    csl = slice(ct * 128, (ct + 1) * 128)
                    ps = lps.tile([128, 512], f32, tag="lt_ps",
                                  name=f"kpps{ct}_{nk}")
                    nc.tensor.matmul(ps, wk8[:, :, csl],
                                     kvn8[:, :, nk * 512:(nk + 1) * 512],
                                     start=True, stop=True, perf_mode=DR)
                    nc.vector.tensor_scalar_mul(
                        kp8[:, ct, nk * 512:(nk + 1) * 512], ps, 1.0 / WS)

            # single persistent [1, 512] denominator bank; chunks reuse it
            # (WAR on the rD read serializes only the chunk seam)
            Dall = dps.tile([1, 512], f32, tag="d_ps", name="Dall")
            pending = None  # (wTs, rDb, q0, N) of the previous chunk

            def drain_applies():
                wTs, rDb, q0, N = pending
                nr, r0 = N // W, q0 // W
                for ct in range(CT):
                    csl = slice(ct * 128, (ct + 1) * 128)
                    a_ps = cps.tile([128, nr, W], f32, tag="a_ps",
                                    name=f"aps{q0}_{ct}")
                    for ktp in range(KT // 2):
                        nc.tensor.matmul(
                            a_ps, vpT_all[:, 2 * ktp:2 * ktp + 2, csl], wTs[ktp],
                            start=(ktp == 0), stop=(ktp == KT // 2 - 1),
                            perf_mode=DR)
                    nc.vector.tensor_mul(a_pad8[:, ct, r0:r0 + nr, 1:W + 1],
                                         a_ps, rDb)

            def conv_block(nk):
                # conv rows 8nk..8nk+7; a_pad rows 8nk..8nk+9 are final.
                # Shares the a-bank psum tag and runs on DVE so the exp
                # stream on ACT is untouched.
                for ct in range(CT):
                    csl = slice(ct * 128, (ct + 1) * 128)
                    ps = cps.tile([128, 8, W], f32, tag="a_ps",
                                  name=f"cps{ct}_{nk}")
                    idx = 0
                    for dy in range(3):
                        for dx in range(3):
                            nc.tensor.matmul(
                                ps, wo8[(dy, dx)][:, :, csl],
                                a_pad8[:, :, 8 * nk + dy:8 * nk + dy + 8,
                                       dx:dx + W],
                                start=(idx == 0), stop=(idx == 8), perf_mode=DR)
                            idx += 1
                    osb = outp.tile([128, 512], bf16, tag="cv_out",
                                    name=f"cvo{ct}_{nk}")
                    nc.vector.tensor_scalar(
                        osb, ps.rearrange("p r w -> p (r w)"), 1.0 / (OS * AS),
                        bo_sb[ct], op0=mybir.AluOpType.mult,
                        op1=mybir.AluOpType.add)
                    nc.sync.dma_start(
                        out_half[ct * 128:(ct + 1) * 128,
                                 nk * 512:(nk + 1) * 512],
                        osb)

            for ci, (q0, N) in enumerate(CHUNKS):
                nr = N // W
                qp8 = attsb.tile([128, 2, N], f8, tag="qp_sb", name=f"qp8_{ci}")
                for ct in range(CT):
                    csl = slice(ct * 128, (ct + 1) * 128)
                    ps = lps.tile([128, N], f32, tag="lt_ps", name=f"qpps{ci}_{ct}")
                    nc.tensor.matmul(ps, wq8[:, :, csl], qn8[:, :, q0:q0 + N],
                                     start=True, stop=True, perf_mode=DR)
                    nc.scalar.activation(qp8[:, ct, :], ps, AF.Identity,
                                         bias=bq_sb[ct], scale=1.0 / WS)
                Dp = Dall[:, 0:N]
                wTs = []
                for ktp in range(KT // 2):
                    if ci == 0 and ktp % 2 == 0:
                        emit_proj_block(ktp // 2)
                    wT8 = wTp.tile([128, 2, N], f8, tag="wT", name=f"wT{ci}_{ktp}")
                    lt2 = lps.tile([128, 2, N], f32, tag="lt_ps",
                                   name=f"lt{ci}_{ktp}")
                    for j in range(2):
                        kt = 2 * ktp + j
                        nc.tensor.matmul(lt2[:, j, :],
                                         kp8[:, :, kt * 128:(kt + 1) * 128],
                                         qp8, start=True, stop=True, perf_mode=DR)
                    if 1 <= ci <= 3 and ktp % 4 == 2:
                        # offload some exps to DVE (Schraudolph bitcast exp,
                        # +-3% -- noise floor is set by fp8 anyway)
                        ti = attsb.tile([128, 2, N], mybir.dt.int32, tag="ei32",
                                        name=f"ei{ci}_{ktp}")
                        nc.vector.tensor_scalar(
                            ti, lt2, EXP_A, EXP_B, op0=mybir.AluOpType.mult,
                            op1=mybir.AluOpType.add)
                        nc.vector.tensor_copy(wT8, ti.bitcast(f32))
                    else:
                        nc.scalar.activation(wT8, lt2, AF.Exp, scale=SC)
                    nc.tensor.matmul(Dp, ones8[:, :, 0:1], wT8, start=(ktp == 0),
                                     stop=(ktp == KT // 2 - 1), perf_mode=DR)
                    wTs.append(wT8)
                rD = attsb.tile([1, N], f32, tag="rD", name=f"rD{ci}")
                nc.vector.reciprocal(rD, Dp)
                nc.vector.tensor_mul(rD, rD, rowmask_sb[0:1, q0:q0 + N])
                rDb = bcp.tile([128, nr, W], f32, tag="rDb", name=f"rDb{ci}")
                nc.gpsimd.partition_broadcast(rDb, rD)
                # apply matmuls run one chunk behind the exp stream so the PE
                # burst never sits between this chunk's exps and the next's
                # logits in the PE queue; conv blocks trail one further chunk
                if pending is not None:
                    drain_applies()
                    if ci >= 2:
                        conv_block(ci - 2)
                pending = (wTs, rDb, q0, N)
            drain_applies()
            conv_block(3)

    nc.compile()
    return nc


def _prep(q, kv, gn_w, gn_b, wq, bq, wkv, bkv, wo, bo):
    q = np.ascontiguousarray(np.asarray(q, np.float32).reshape(B, C, HW))
    kv = np.ascontiguousarray(np.asarray(kv, np.float32).reshape(B, C, HW))
    wq = np.asarray(wq, np.float32)
    wkv = np.asarray(wkv, np.float32)
    wo = np.asarray(wo, np.float32)
    wk = wkv[0::2]
    wv = wkv[1::2]
    bk = np.asarray(bkv, np.float32)[0::2]
    bv = np.asarray(bkv, np.float32)[1::2]

    p = np.arange(128)
    gmask = np.zeros((128, GPT), np.float32)
    gmask[p, p // 8] = 1.0 / 8.0
    bmask = np.zeros((GPT, 128), np.float32)
    bmask[p // 8, p] = 1.0

    cols = np.stack([
        np.asarray(gn_w, np.float32), np.asarray(gn_b, np.float32),
        np.asarray(bq, np.float32), np.asarray(bo, np.float32)], axis=1)
    woT = wo.transpose(1, 2, 3, 0).reshape(C, 9 * C)  # [ci, (dy dx co)]
    wpack = np.concatenate([wq.T * WS, wk.T * WS, wv.T * WS, woT * OS], axis=1)
    wpack = np.clip(wpack, -240.0, 240.0)
    wpack8 = wpack.astype(F8).reshape(2, 128, 12 * C).transpose(1, 0, 2)
    common = {
        "wpack": np.ascontiguousarray(wpack8),
        "cols": np.ascontiguousarray(cols),
        "gmask": gmask,
        "bmask": bmask,
    }

    # bv enters the output linearly: a = a_nobias + bv[c]  =>
    # out += conv3x3(bv_map) with SAME zero padding. Precomputed here and
    # added with the host residual. (bk is a softmax no-op and is dropped.)
    tap = np.einsum("oikl,i->okl", wo, bv)  # [C_out, 3, 3]
    bias_map = np.zeros((C, H, W), np.float32)
    for dy in range(3):
        for dx in range(3):
            y0, y1 = max(0, 1 - dy), min(H, H + 1 - dy)
            x0, x1 = max(0, 1 - dx), min(W, W + 1 - dx)
            bias_map[:, y0:y1, x0:x1] += tap[:, dy, dx][:, None, None]

    q_bf = q.astype(BF16)
    kv_bf = kv.astype(BF16)
    in_maps = []
    for core in range(8):
        b, top = core // 2, core % 2 == 0
        qimg = q_bf[b].reshape(C, H, W)
        q34 = np.zeros((C, NROWS, W), BF16)
        mask = np.full((NROWS, W), AS * SC, np.float32)
        if top:
            q34[:, 1:34] = qimg[:, 0:33]
            mask[0] = 0.0
        else:
            q34[:, 0:33] = qimg[:, 31:64]
            mask[33] = 0.0
        in_maps.append({
            **common,
            "q_full": q_bf[b],
            "kv_full": kv_bf[b],
            "q34": np.ascontiguousarray(q34.reshape(C, NQ)),
            "rowmask": np.ascontiguousarray(mask.reshape(1, NQ)),
        })
    return in_maps, bias_map


def _make_runner(nc, n_cores=8):
    """Cached variant of bass2jax.run_bass_via_pjrt: builds the sharded jit
    once so repeated kernel() calls skip retracing the program."""
    import jax
    import numpy as _np
    from jax.sharding import Mesh, PartitionSpec
    from jax.experimental.shard_map import shard_map
    from concourse import mybir
    from concourse.bass2jax import (_bass_exec_p, install_neuronx_cc_hook,
                                    partition_id_tensor)

    install_neuronx_cc_hook()

    partition_name = nc.partition_id_tensor.name if nc.partition_id_tensor else None
    in_names, out_names, out_avals, zero_outs = [], [], [], []
    for alloc in nc.m.functions[0].allocations:
        if not isinstance(alloc, mybir.MemoryLocationSet):
            continue
        name = alloc.memorylocations[0].name
        if alloc.kind == "ExternalInput":
            if name != partition_name:
                in_names.append(name)
        elif alloc.kind == "ExternalOutput":
            shape = tuple(alloc.tensor_shape)
            np_dt = mybir.dt.np(alloc.dtype)
            out_names.append(name)
            out_avals.append(jax.core.ShapedArray(shape, np_dt))
            zero_outs.append(_np.zeros(shape, np_dt))

    n_params = len(in_names)
    n_outs = len(out_names)
    all_in_names = in_names + out_names
    if partition_name is not None:
        all_in_names.append(partition_name)
    donate = tuple(range(n_params, n_params + n_outs))

    def _body(*args):
        operands = list(args)
        if partition_name is not None:
            operands.append(partition_id_tensor())
        outs = _bass_exec_p.bind(
            *operands,
            out_avals=tuple(out_avals),
            in_names=tuple(all_in_names),
            out_names=tuple(out_names),
            lowering_input_output_aliases=(),
            sim_require_finite=True,
            sim_require_nnan=True,
            nc=nc,
        )
        return tuple(outs)

    devices = jax.devices()[:n_cores]
    mesh = Mesh(_np.asarray(devices), ("core",))
    in_specs = (PartitionSpec("core"),) * (n_params + n_outs)
    out_specs = (PartitionSpec("core"),) * n_outs
    sharded = jax.jit(
        shard_map(_body, mesh=mesh, in_specs=in_specs, out_specs=out_specs,
                  check_rep=False),
        donate_argnums=donate, keep_unused=True)

    import jax.numpy as jnp
    from jax.sharding import NamedSharding
    out_shard = NamedSharding(mesh, PartitionSpec("core"))

    def run(in_maps):
        concat_in = [
            _np.concatenate([_np.asarray(m[name]) for m in in_maps], axis=0)
            for name in in_names
        ]
        # donation buffers created directly on device — nothing to upload
        concat_zeros = [
            jnp.zeros((n_cores * z.shape[0], *z.shape[1:]), z.dtype,
                      device=out_shard)
            for z in zero_outs
        ]
        out_arrs = sharded(*concat_in, *concat_zeros)
        return [
            {name: _np.asarray(out_arrs[i]).reshape(n_cores, *out_avals[i].shape)[c]
             for i, name in enumerate(out_names)}
            for c in range(n_cores)
        ]

    return run


def kernel(q, kv, gn_w, gn_b, wq, bq, wkv, bkv, wo, bo):
    if "run" not in _CACHE:
        nc = _build()
        _CACHE["run"] = _make_runner(nc)
    in_maps, bias_map = _prep(q, kv, gn_w, gn_b, wq, bq, wkv, bkv, wo, bo)
    res = _CACHE["run"](in_maps)
    out = np.empty((B, C, H, W), np.float32)
    qf = np.asarray(q, np.float32)
    for core in range(8):
        b, r0 = core // 2, 0 if core % 2 == 0 else 32
        # residual (+ v-bias conv map) added on host in fp32; the device ships
        # only the tiny attention/conv delta
        out[b, :, r0:r0 + 32, :] = (
            res[core]["out_half"].astype(np.float32).reshape(C, 32, W)
            + qf[b, :, r0:r0 + 32, :] + bias_map[:, r0:r0 + 32, :])
    return out



# revision 8
# speedup vs baseline: 2.3096x; 2.3096x over previous
"""Trainium2 Bass kernel for CrossAttentionBlock (GN -> qkv proj -> full attention -> conv3x3; fp32 residual on host).

The wall-clock budget is dominated by the axon relay (~44 MB/s each way,
~80 ms per dispatch), so the design minimizes transferred bytes:

  - GroupNorm statistics are computed on host (cheap fp32 numpy); the device
    receives RAW fp8 activations plus per-channel (scale, bias) columns and
    applies the normalize during the fp8->fp8 activation copy.
  - Per core upload: its 34 query rows (32 + conv halo) and its sample's full
    kv, packed fp8 in DoubleRow layout [128, 2, n] -> 1.6 MB/core, 12.9 MB
    total (vs 49 MB for the bf16 full-tensor scheme).
  - Static tensors (fp8 weight pack, softmax row mask) are uploaded once and
    cached as committed device arrays across calls; a fingerprint check
    re-uploads if the weights actually change.
  - The output is the tiny conv delta scaled by 2^18 in fp8 (0.5 MB/core);
    the fp32 residual (+q), the v-bias conv map and bo are added on host.
  - Output zero-buffers are created inside the jit body (no extra dispatch),
    and the q+bias host work overlaps the device round trip.

Sharding: 8 cores = 4 samples x 2 query-row-halves; each core computes
attention for 34 query rows against all 4096 keys, then conv3x3 for its 32
rows. All heavy matmuls run in fp8e4m3 DoubleRow (2 MACs/cell/cycle):
  - wq/wk/wv pre-scaled x16 on host (raw std ~0.028 subnormal in fp8); copies
    out of PSUM descale by 1/16.
  - attention 1/sqrt(C) lives in the Exp activation scale.
  - v path keeps the x16; rowmask carries 4.0 = 64/16 so softmax-normalize
    produces a_pad = 64*a.
  - wo pre-scaled x2^22; conv PSUM copy scales by 2^18/(2^22*64) so the fp8
    output carries 2^18*delta.
"""

import sys

if "/opt/trn_rl_repo" not in sys.path:
    sys.path.insert(0, "/opt/trn_rl_repo")

import ml_dtypes
import numpy as np

B, C, H, W = 4, 256, 64, 64
HW = H * W              # 4096
CT = C // 128           # 2 channel partition-tiles
KT = HW // 128          # 32 key tiles
EPS = 1e-5
NROWS = 34              # 32 output rows + halo row each side
NQ = NROWS * W          # 2176 queries per core
NOUT = 32 * W           # 2048 outputs per core
QKVN = NQ + HW          # packed activation width per core
CHUNKS = [(0, 512), (512, 512), (1024, 512), (1536, 512), (2048, 128)]
BF16 = ml_dtypes.bfloat16
F8 = ml_dtypes.float8_e4m3
WS = 16.0               # host pre-scale on wq/wk/wv
OS = float(2 ** 22)     # host pre-scale on wo
AS = 64.0               # a_pad carries 64*a
OSC = float(2 ** 18)    # fp8 output carries OSC * conv-delta
SC = 1.0 / 16.0         # attention 1/sqrt(C), applied inside Exp
EXP_A = (2.0 ** 23) / float(np.log(2.0)) * SC   # Schraudolph exp slope
EXP_B = float(127 * 2 ** 23 - 486411)           # Schraudolph exp bias

_CACHE = {}


def _build():
    import concourse.bass as bass
    import concourse.tile as tile
    from concourse import bacc, mybir

    f32 = mybir.dt.float32
    f8 = mybir.dt.float8e4
    AF = mybir.ActivationFunctionType
    DR = mybir.MatmulPerfMode.DoubleRow

    nc = bacc.Bacc("TRN2", target_bir_lowering=False)

    # dynamic per-call inputs
    qkv_d = nc.dram_tensor("qkv8", [128, 2, QKVN], f8, kind="ExternalInput")
    # per-channel columns [p, j, col]: scol_q, bcol_q, scol_kv, bcol_kv, bq
    cols_d = nc.dram_tensor("cols", [128, 2, 5], f32, kind="ExternalInput")
    # static (device-cached) inputs
    # packed fp8 weights, DoubleRow layout [p, j, cols]; c_in = p + 128*j.
    # column blocks: [wqT | wkT | wvT | woT(dy,dx) x 9] each C wide
    wpack_d = nc.dram_tensor("wpack", [128, 2, 12 * C], f8, kind="ExternalInput")
    rowmask_d = nc.dram_tensor("rowmask", [1, NQ], f32, kind="ExternalInput")
    out_d = nc.dram_tensor("out8", [C, NOUT], f8, kind="ExternalOutput")

    with tile.TileContext(nc) as tc, \
         tc.tile_pool(name="const", bufs=1) as constp, \
         tc.tile_pool(name="acts", bufs=1) as acts:

        # ---------------- input DMAs (sync queue order = priority) ----------
        raw8 = constp.tile([128, 2, QKVN], f8, tag="raw8", name="raw8")
        for d in range(4):
            s = d * (QKVN // 4)
            nc.sync.dma_start(raw8[:, :, s:s + QKVN // 4],
                              qkv_d[:, :, s:s + QKVN // 4])
        cols_sb = constp.tile([128, 2, 5], f32, tag="cols", name="cols_sb")
        nc.sync.dma_start(cols_sb, cols_d[:, :, :])
        wpack_sb = constp.tile([128, 2, 12 * C], f8, tag="wpack", name="wpack_sb")
        nc.sync.dma_start(wpack_sb, wpack_d[:, :, :])

        def blk(i):
            return wpack_sb[:, :, i * C:(i + 1) * C]

        wq8, wk8, wv8 = blk(0), blk(1), blk(2)
        wo8 = {(dy, dx): blk(3 + dy * 3 + dx) for dy in range(3) for dx in range(3)}
        bq_sb = [cols_sb[:, ct, 4:5] for ct in range(CT)]

        rowmask_sb = constp.tile([1, NQ], f32, tag="rowmask", name="rowmask_sb")
        nc.gpsimd.dma_start(rowmask_sb, rowmask_d[:, :])
        # [128, 2, 16] so the DoubleRow pair-step is 16 B (s3_lw_dual_fp8)
        ones8 = constp.tile([128, 2, 16], f8, tag="ones8", name="ones8")
        nc.vector.memset(ones8, 1.0)

        # ---------------- persistent activations (fp8 DoubleRow layouts) ----
        kvn8 = acts.tile([128, 2, HW], f8, tag="kvn8", name="kvn8")
        qn8 = acts.tile([128, 2, NQ], f8, tag="qn8", name="qn8")
        kp8 = acts.tile([128, 2, HW], f8, tag="kp8", name="kp8")
        vpT_all = acts.tile([128, KT, C], f8, tag="vpT", name="vpT_all")
        a_pad8 = acts.tile([128, 2, NROWS, W + 2], f8, tag="a_pad", name="a_pad8")
        nc.gpsimd.memset(a_pad8, 0.0)

        # ---------------- GroupNorm apply (stats computed on host) ----------
        for j in range(CT):
            nc.scalar.activation(qn8[:, j, :], raw8[:, j, 0:NQ], AF.Identity,
                                 bias=cols_sb[:, j, 1:2], scale=cols_sb[:, j, 0:1])
            nc.scalar.activation(kvn8[:, j, :], raw8[:, j, NQ:QKVN], AF.Identity,
                                 bias=cols_sb[:, j, 3:4], scale=cols_sb[:, j, 2:3])

        # ---------------- projections + attention ----------------
        # One PSUM budget for both phases (D 1 + lt 3x2 + a 1 = 8 banks).
        # Proj psum tiles ride the lt-slot rotation, emitted inside chunk 0's
        # kt loop right before the lt that consumes them, so attention starts
        # immediately and the proj copies drain on DVE behind the exp stream.
        with tc.tile_pool(name="d_ps", bufs=1, space="PSUM") as dps, \
             tc.tile_pool(name="att_lt", bufs=3, space="PSUM") as lps, \
             tc.tile_pool(name="acc_ps", bufs=1, space="PSUM") as cps, \
             tc.tile_pool(name="attsb", bufs=3) as attsb, \
             tc.tile_pool(name="wTp", bufs=34) as wTp, \
             tc.tile_pool(name="bcast", bufs=2) as bcp, \
             tc.tile_pool(name="outp", bufs=3) as outp:

            def emit_proj_block(nk):
                for ht in (4 * nk, 4 * nk + 1, 4 * nk + 2, 4 * nk + 3):
                    ps = lps.tile([128, C], f32, tag="lt_ps", name=f"vpps{ht}")
                    nc.tensor.matmul(ps, kvn8[:, :, ht * 128:(ht + 1) * 128], wv8,
                                     start=True, stop=True, perf_mode=DR)
                    nc.vector.tensor_copy(vpT_all[:, ht, :], ps)
                for ct in range(CT):
                    csl = slice(ct * 128, (ct + 1) * 128)
                    ps = lps.tile([128, 512], f32, tag="lt_ps",
                                  name=f"kpps{ct}_{nk}")
                    nc.tensor.matmul(ps, wk8[:, :, csl],
                                     kvn8[:, :, nk * 512:(nk + 1) * 512],
                                     start=True, stop=True, perf_mode=DR)
                    nc.vector.tensor_scalar_mul(
                        kp8[:, ct, nk * 512:(nk + 1) * 512], ps, 1.0 / WS)

            # single persistent [1, 512] denominator bank; chunks reuse it
            # (WAR on the rD read serializes only the chunk seam)
            Dall = dps.tile([1, 512], f32, tag="d_ps", name="Dall")
            pending = None  # (wTs, rDb, q0, N) of the previous chunk

            def drain_applies():
                wTs, rDb, q0, N = pending
                nr, r0 = N // W, q0 // W
                for ct in range(CT):
                    csl = slice(ct * 128, (ct + 1) * 128)
                    a_ps = cps.tile([128, nr, W], f32, tag="a_ps",
                                    name=f"aps{q0}_{ct}")
                    for ktp in range(KT // 2):
                        nc.tensor.matmul(
                            a_ps, vpT_all[:, 2 * ktp:2 * ktp + 2, csl], wTs[ktp],
                            start=(ktp == 0), stop=(ktp == KT // 2 - 1),
                            perf_mode=DR)
                    nc.vector.tensor_mul(a_pad8[:, ct, r0:r0 + nr, 1:W + 1],
                                         a_ps, rDb)

            def conv_block(nk):
                # conv rows 8nk..8nk+7; a_pad rows 8nk..8nk+9 are final.
                # Shares the a-bank psum tag and runs on DVE so the exp
                # stream on ACT is untouched.
                for ct in range(CT):
                    csl = slice(ct * 128, (ct + 1) * 128)
                    ps = cps.tile([128, 8, W], f32, tag="a_ps",
                                  name=f"cps{ct}_{nk}")
                    idx = 0
                    for dy in range(3):
                        for dx in range(3):
                            nc.tensor.matmul(
                                ps, wo8[(dy, dx)][:, :, csl],
                                a_pad8[:, :, 8 * nk + dy:8 * nk + dy + 8,
                                       dx:dx + W],
                                start=(idx == 0), stop=(idx == 8), perf_mode=DR)
                            idx += 1
                    osb = outp.tile([128, 512], f8, tag="cv_out",
                                    name=f"cvo{ct}_{nk}")
                    nc.vector.tensor_scalar_mul(
                        osb, ps.rearrange("p r w -> p (r w)"), OSC / (OS * AS))
                    nc.sync.dma_start(
                        out_d[ct * 128:(ct + 1) * 128,
                              nk * 512:(nk + 1) * 512],
                        osb)

            for ci, (q0, N) in enumerate(CHUNKS):
                nr = N // W
                qp8 = attsb.tile([128, 2, N], f8, tag="qp_sb", name=f"qp8_{ci}")
                for ct in range(CT):
                    csl = slice(ct * 128, (ct + 1) * 128)
                    ps = lps.tile([128, N], f32, tag="lt_ps", name=f"qpps{ci}_{ct}")
                    nc.tensor.matmul(ps, wq8[:, :, csl], qn8[:, :, q0:q0 + N],
                                     start=True, stop=True, perf_mode=DR)
                    nc.scalar.activation(qp8[:, ct, :], ps, AF.Identity,
                                         bias=bq_sb[ct], scale=1.0 / WS)
                Dp = Dall[:, 0:N]
                wTs = []
                for ktp in range(KT // 2):
                    if ci == 0 and ktp % 2 == 0:
                        emit_proj_block(ktp // 2)
                    wT8 = wTp.tile([128, 2, N], f8, tag="wT", name=f"wT{ci}_{ktp}")
                    lt2 = lps.tile([128, 2, N], f32, tag="lt_ps",
                                   name=f"lt{ci}_{ktp}")
                    for j in range(2):
                        kt = 2 * ktp + j
                        nc.tensor.matmul(lt2[:, j, :],
                                         kp8[:, :, kt * 128:(kt + 1) * 128],
                                         qp8, start=True, stop=True, perf_mode=DR)
                    if 1 <= ci <= 3 and ktp % 4 == 2:
                        # offload some exps to DVE (Schraudolph bitcast exp,
                        # +-3% -- noise floor is set by fp8 anyway)
                        ti = attsb.tile([128, 2, N], mybir.dt.int32, tag="ei32",
                                        name=f"ei{ci}_{ktp}")
                        nc.vector.tensor_scalar(
                            ti, lt2, EXP_A, EXP_B, op0=mybir.AluOpType.mult,
                            op1=mybir.AluOpType.add)
                        nc.vector.tensor_copy(wT8, ti.bitcast(f32))
                    else:
                        nc.scalar.activation(wT8, lt2, AF.Exp, scale=SC)
                    nc.tensor.matmul(Dp, ones8[:, :, 0:1], wT8, start=(ktp == 0),
                                     stop=(ktp == KT // 2 - 1), perf_mode=DR)
                    wTs.append(wT8)
                rD = attsb.tile([1, N], f32, tag="rD", name=f"rD{ci}")
                nc.vector.reciprocal(rD, Dp)
                nc.vector.tensor_mul(rD, rD, rowmask_sb[0:1, q0:q0 + N])
                rDb = bcp.tile([128, nr, W], f32, tag="rDb", name=f"rDb{ci}")
                nc.gpsimd.partition_broadcast(rDb, rD)
                # apply matmuls run one chunk behind the exp stream so the PE
                # burst never sits between this chunk's exps and the next's
                # logits in the PE queue; conv blocks trail one further chunk
                if pending is not None:
                    drain_applies()
                    if ci >= 2:
                        conv_block(ci - 2)
                pending = (wTs, rDb, q0, N)
            drain_applies()
            conv_block(3)

    nc.compile()
    return nc


def _make_runner(nc, n_cores=8):
    """Builds a cached jit of the bass program. Output zero-buffers are
    created inside the jit body (no separate device allocation dispatch);
    the kernel writes every output element so their content is never read."""
    import jax
    import jax.numpy as jnp
    import numpy as _np
    from jax.sharding import Mesh, PartitionSpec, NamedSharding
    from jax.experimental.shard_map import shard_map
    from concourse import mybir
    from concourse.bass2jax import (_bass_exec_p, install_neuronx_cc_hook,
                                    partition_id_tensor)

    install_neuronx_cc_hook()

    partition_name = nc.partition_id_tensor.name if nc.partition_id_tensor else None
    in_names, out_names, out_avals = [], [], []
    for alloc in nc.m.functions[0].allocations:
        if not isinstance(alloc, mybir.MemoryLocationSet):
            continue
        name = alloc.memorylocations[0].name
        if alloc.kind == "ExternalInput":
            if name != partition_name:
                in_names.append(name)
        elif alloc.kind == "ExternalOutput":
            shape = tuple(alloc.tensor_shape)
            np_dt = mybir.dt.np(alloc.dtype)
            out_names.append(name)
            out_avals.append(jax.core.ShapedArray(shape, np_dt))

    n_params = len(in_names)
    all_in_names = in_names + out_names
    if partition_name is not None:
        all_in_names.append(partition_name)

    def _body(*args):
        operands = list(args)
        if partition_name is not None:
            operands.append(partition_id_tensor())
        outs = _bass_exec_p.bind(
            *operands,
            out_avals=tuple(out_avals),
            in_names=tuple(all_in_names),
            out_names=tuple(out_names),
            lowering_input_output_aliases=(),
            sim_require_finite=True,
            sim_require_nnan=True,
            nc=nc,
        )
        return tuple(outs)

    devices = jax.devices()[:n_cores]
    mesh = Mesh(_np.asarray(devices), ("core",))
    n_outs = len(out_names)
    in_specs = (PartitionSpec("core"),) * (n_params + n_outs)
    out_specs = (PartitionSpec("core"),) * n_outs
    # The out buffers are passed as cached NON-donated zero inputs: the
    # kernel writes every output element, so their content is never read and
    # one committed device array can be reused across calls (no per-call
    # allocation dispatch, no transfer).
    sharded = jax.jit(
        shard_map(_body, mesh=mesh, in_specs=in_specs, out_specs=out_specs,
                  check_rep=False))
    shard = NamedSharding(mesh, PartitionSpec("core"))
    import jax.numpy as _jnp
    zero_devs = [
        _jnp.zeros((n_cores * a.shape[0], *a.shape[1:]), a.dtype, device=shard)
        for a in out_avals
    ]
    return sharded, shard, in_names, out_names, zero_devs


def _pack_static(wq, bq, wkv, bkv, wo, gn_w, gn_b, bo):
    """Device-static arrays (weight pack, rowmask) + host-side bias map."""
    wq = np.asarray(wq, np.float32)
    wkv = np.asarray(wkv, np.float32)
    wo = np.asarray(wo, np.float32)
    wk = wkv[0::2]
    wv = wkv[1::2]
    bv = np.asarray(bkv, np.float32)[1::2]

    woT = wo.transpose(1, 2, 3, 0).reshape(C, 9 * C)  # [ci, (dy dx co)]
    wpack = np.concatenate([wq.T * WS, wk.T * WS, wv.T * WS, woT * OS], axis=1)
    wpack = np.clip(wpack, -240.0, 240.0)
    wpack8 = wpack.astype(F8).reshape(2, 128, 12 * C).transpose(1, 0, 2)
    # replicate per core and flatten the core axis into the shard axis
    wpack8 = np.ascontiguousarray(
        np.broadcast_to(wpack8, (8, 128, 2, 12 * C))).reshape(8 * 128, 2, 12 * C)

    # rowmask: AS*SC softmax scaling, zeroed on the out-of-image halo row
    rowmask = np.empty((8, NQ), np.float32)
    for core in range(8):
        m = np.full((NROWS, W), AS * SC, np.float32)
        if core % 2 == 0:
            m[0] = 0.0
        else:
            m[NROWS - 1] = 0.0
        rowmask[core] = m.reshape(NQ)

    # bv enters the output linearly: a = a_nobias + bv[c]  =>
    # out += conv3x3(bv_map) with SAME zero padding; bo is added here too.
    # (bk is a softmax no-op and is dropped.)
    tap = np.einsum("oikl,i->okl", wo, bv)  # [C_out, 3, 3]
    bias_map = np.zeros((C, H, W), np.float32)
    for dy in range(3):
        for dx in range(3):
            y0, y1 = max(0, 1 - dy), min(H, H + 1 - dy)
            x0, x1 = max(0, 1 - dx), min(W, W + 1 - dx)
            bias_map[:, y0:y1, x0:x1] += tap[:, dy, dx][:, None, None]
    bias_map += np.asarray(bo, np.float32)[:, None, None]
    return wpack8, rowmask, bias_map


def kernel(q, kv, gn_w, gn_b, wq, bq, wkv, bkv, wo, bo):
    import jax

    if "run" not in _CACHE:
        nc = _build()
        _CACHE["run"] = _make_runner(nc)
    sharded, shard, in_names, out_names, zero_devs = _CACHE["run"]

    q = np.asarray(q, np.float32).reshape(B, C, HW)
    kv = np.asarray(kv, np.float32).reshape(B, C, HW)

    # ---- static (weight) arrays: cache committed device buffers ----
    wlist = (wq, bq, wkv, bkv, wo, bo, gn_w, gn_b)
    st = _CACHE.get("static")
    if st is None or not all(
            np.array_equal(np.asarray(a, np.float32), b)
            for a, b in zip(wlist, st["wlist"])):
        wpack8, rowmask, bias_map = _pack_static(
            wq, bq, wkv, bkv, wo, gn_w, gn_b, bo)
        st = {
            "wlist": [np.array(np.asarray(a, np.float32)) for a in wlist],
            "bias_map": bias_map,
            "wpack_dev": jax.device_put(np.ascontiguousarray(wpack8), shard),
            "rowmask_dev": jax.device_put(rowmask, shard),
        }
        _CACHE["static"] = st

    # ---- dynamic prep: GN stats on host, raw fp8 in DoubleRow layout ----
    gw = np.asarray(gn_w, np.float32)
    gb = np.asarray(gn_b, np.float32)
    bqv = np.asarray(bq, np.float32)
    cols = np.empty((B, C, 5), np.float32)
    for xi, x in enumerate((q, kv)):
        xg = x.reshape(B, 32, 8 * HW)
        m = xg.mean(axis=2)
        v = xg.var(axis=2)
        rstd = 1.0 / np.sqrt(v + EPS)           # [B, 32]
        scol = gw[None, :] * np.repeat(rstd, 8, axis=1)    # [B, C]
        bcol = gb[None, :] - np.repeat(m, 8, axis=1) * scol
        cols[:, :, 2 * xi] = scol
        cols[:, :, 2 * xi + 1] = bcol
    cols[:, :, 4] = bqv[None, :]
    # [B, C, 5] -> per-core [128, 2, 5] with c = p + 128*j
    cols_pc = cols.reshape(B, 2, 128, 5).transpose(0, 2, 1, 3)   # [B,128,2,5]
    cols_up = np.repeat(cols_pc, 2, axis=0)                      # [8,128,2,5]

    # kv first so its transfer can start while q is being packed
    kv8 = kv.astype(F8).reshape(B, 2, 128, HW).transpose(0, 2, 1, 3)
    U = np.zeros((8, 128, 2, QKVN), F8)
    U[0::2, :, :, NQ:] = kv8
    U[1::2, :, :, NQ:] = kv8
    q8 = q.reshape(B, C, H, W).astype(F8).reshape(B, 2, 128, H, W)
    qt = q8.transpose(0, 2, 1, 3, 4)                 # [B,128,2,H,W]
    U[0::2, :, :, W:NQ] = qt[:, :, :, 0:NROWS - 1].reshape(B, 128, 2, NQ - W)
    U[1::2, :, :, 0:NQ - W] = qt[:, :, :, H - NROWS + 1:H].reshape(
        B, 128, 2, NQ - W)

    arrs = {
        "qkv8": jax.device_put(U.reshape(8 * 128, 2, QKVN), shard),
        "cols": jax.device_put(
            np.ascontiguousarray(cols_up).reshape(8 * 128, 2, 5), shard),
        "wpack": st["wpack_dev"],
        "rowmask": st["rowmask_dev"],
    }
    fut = sharded(*[arrs[n] for n in in_names], *zero_devs)

    # host residual overlaps the device round trip
    base = q.reshape(B, C, H, W) + st["bias_map"][None]
    delta = np.asarray(fut[0]).astype(np.float32) * (1.0 / OSC)
    delta = delta.reshape(8, C, 32, W)
    out = base
    for core in range(8):
        b, r0 = core // 2, 0 if core % 2 == 0 else 32
        out[b, :, r0:r0 + 32, :] += delta[core]
    return out


# revision 14
# speedup vs baseline: 3.4011x; 1.4726x over previous
"""Trainium2 Bass kernel for CrossAttentionBlock (GN -> qkv proj -> full attention -> conv3x3; fp32 residual on host).

The wall-clock budget is dominated by the axon relay (~44 MB/s each way,
~80 ms per dispatch), so the design minimizes transferred bytes:

  - GroupNorm statistics are computed on host (cheap fp32 numpy); the device
    receives RAW fp8 activations plus per-channel (scale, bias) columns and
    applies the normalize during the fp8->fp8 activation copy.
  - Per core upload: its 34 query rows (32 + conv halo) and its sample's full
    kv, packed fp8 in DoubleRow layout [128, 2, n] -> 1.6 MB/core, 12.9 MB
    total (vs 49 MB for the bf16 full-tensor scheme).
  - Static tensors (fp8 weight pack, softmax row mask) are uploaded once and
    cached as committed device arrays across calls; a fingerprint check
    re-uploads if the weights actually change.
  - The output is the tiny conv delta scaled by 2^18 in fp8 (0.5 MB/core);
    the fp32 residual (+q), the v-bias conv map and bo are added on host.
  - Output zero-buffers are created inside the jit body (no extra dispatch),
    and the q+bias host work overlaps the device round trip.

Sharding: 8 cores = 4 samples x 2 query-row-halves; each core computes
attention for 34 query rows against all 4096 keys, then conv3x3 for its 32
rows. All heavy matmuls run in fp8e4m3 DoubleRow (2 MACs/cell/cycle):
  - wq/wk/wv pre-scaled x16 on host (raw std ~0.028 subnormal in fp8); copies
    out of PSUM descale by 1/16.
  - attention 1/sqrt(C) lives in the Exp activation scale.
  - v path keeps the x16; rowmask carries 4.0 = 64/16 so softmax-normalize
    produces a_pad = 64*a.
  - wo pre-scaled x2^22; conv PSUM copy scales by 2^18/(2^22*64) so the fp8
    output carries 2^18*delta.
"""

import sys

if "/opt/trn_rl_repo" not in sys.path:
    sys.path.insert(0, "/opt/trn_rl_repo")

import ml_dtypes
import numpy as np

B, C, H, W = 4, 256, 64, 64
HW = H * W              # 4096
CT = C // 128           # 2 channel partition-tiles
KT = HW // 128          # 32 key tiles
EPS = 1e-5
NROWS = 34              # 32 output rows + halo row each side
NQ = NROWS * W          # 2176 queries per core
NOUT = 32 * W           # 2048 outputs per core
QKVN = NQ + HW          # packed activation width per core
CHUNKS = [(0, 512), (512, 512), (1024, 512), (1536, 512), (2048, 128)]
BF16 = ml_dtypes.bfloat16
F8 = ml_dtypes.float8_e4m3
WS = 16.0               # host pre-scale on wq/wk/wv
OS = float(2 ** 22)     # host pre-scale on wo
AS = 64.0               # a_pad carries 64*a
OSC = float(2 ** 18)    # fp8 output carries OSC * conv-delta
SC = 1.0 / 16.0         # attention 1/sqrt(C), applied inside Exp
EXP_A = (2.0 ** 23) / float(np.log(2.0)) * SC   # Schraudolph exp slope
EXP_B = float(127 * 2 ** 23 - 486411)           # Schraudolph exp bias

_CACHE = {}


def _build():
    import concourse.bass as bass
    import concourse.tile as tile
    from concourse import bacc, mybir

    f32 = mybir.dt.float32
    f8 = mybir.dt.float8e4
    u8 = mybir.dt.uint8
    i32 = mybir.dt.int32
    AF = mybir.ActivationFunctionType
    DR = mybir.MatmulPerfMode.DoubleRow
    ALU = mybir.AluOpType

    nc = bacc.Bacc("TRN2", target_bir_lowering=False)

    # dynamic per-call inputs: int4-packed raw activations (lo nibble = first
    # half of the flat column range, hi nibble = second half)
    kv4_d = nc.dram_tensor("kv4", [128, 2, HW // 2], u8, kind="ExternalInput")
    q4_d = nc.dram_tensor("q4", [128, 2, NQ // 2], u8, kind="ExternalInput")
    # per-channel columns [p, j, col]: scol_q, bcol_q, scol_kv, bcol_kv, bq
    # (scol/bcol fold the GN stats, the gn affine, and the int4 grid scale)
    cols_d = nc.dram_tensor("cols", [128, 2, 5], f32, kind="ExternalInput")
    # static (device-cached) inputs
    # packed fp8 weights, DoubleRow layout [p, j, cols]; c_in = p + 128*j.
    # column blocks: [wqT | wkT | wvT | woT(dy,dx) x 9] each C wide
    wpack_d = nc.dram_tensor("wpack", [128, 2, 12 * C], f8, kind="ExternalInput")
    rowmask_d = nc.dram_tensor("rowmask", [1, NQ], f32, kind="ExternalInput")
    out_d = nc.dram_tensor("out8", [C, NOUT], f8, kind="ExternalOutput")

    with tile.TileContext(nc) as tc, \
         tc.tile_pool(name="const", bufs=1) as constp, \
         tc.tile_pool(name="acts", bufs=1) as acts:

        # ---------------- input DMAs (sync queue order = priority) ----------
        raw_kv = constp.tile([128, 2, HW // 2], u8, tag="rawkv", name="raw_kv")
        for d in range(2):
            s = d * (HW // 4)
            nc.sync.dma_start(raw_kv[:, :, s:s + HW // 4],
                              kv4_d[:, :, s:s + HW // 4])
        raw_q = constp.tile([128, 2, NQ // 2], u8, tag="rawq", name="raw_q")
        nc.sync.dma_start(raw_q, q4_d[:, :, :])
        cols_sb = constp.tile([128, 2, 5], f32, tag="cols", name="cols_sb")
        nc.sync.dma_start(cols_sb, cols_d[:, :, :])
        wpack_sb = constp.tile([128, 2, 12 * C], f8, tag="wpack", name="wpack_sb")
        nc.sync.dma_start(wpack_sb, wpack_d[:, :, :])

        def blk(i):
            return wpack_sb[:, :, i * C:(i + 1) * C]

        wq8, wk8, wv8 = blk(0), blk(1), blk(2)
        wo8 = {(dy, dx): blk(3 + dy * 3 + dx) for dy in range(3) for dx in range(3)}
        bq_sb = [cols_sb[:, ct, 4:5] for ct in range(CT)]

        rowmask_sb = constp.tile([1, NQ], f32, tag="rowmask", name="rowmask_sb")
        nc.gpsimd.dma_start(rowmask_sb, rowmask_d[:, :])
        # [128, 2, 16] so the DoubleRow pair-step is 16 B (s3_lw_dual_fp8)
        ones8 = constp.tile([128, 2, 16], f8, tag="ones8", name="ones8")
        nc.vector.memset(ones8, 1.0)

        # ---------------- persistent activations (fp8 DoubleRow layouts) ----
        kvn8 = acts.tile([128, 2, HW], f8, tag="kvn8", name="kvn8")
        qn8 = acts.tile([128, 2, NQ], f8, tag="qn8", name="qn8")
        kp8 = acts.tile([128, 2, HW], f8, tag="kp8", name="kp8")
        vpT_all = acts.tile([128, KT, C], f8, tag="vpT", name="vpT_all")
        a_pad8 = acts.tile([128, 2, NROWS, W + 2], f8, tag="a_pad", name="a_pad8")
        nc.gpsimd.memset(a_pad8, 0.0)

        # ------------- int4 unpack + GroupNorm apply (stats on host) --------
        # n (0..15) -> xhat = (n - 7.5) * s; normalize = scol*xhat + bcol,
        # both folded into the cols scale/bias on host.
        with tc.tile_pool(name="unpk", bufs=1) as up:
            t32 = up.tile([128, HW // 2], i32, tag="t32", name="t32")
            t32b = up.tile([128, HW // 2], i32, tag="t32b", name="t32b")
            tf = up.tile([128, HW // 2], f32, tag="tf", name="tf")
            for j in range(CT):
                for raw, M, dst, sc in ((raw_kv, HW, kvn8, 2),
                                        (raw_q, NQ, qn8, 0)):
                    h = M // 2
                    scol = cols_sb[:, j, sc:sc + 1]
                    bcol = cols_sb[:, j, sc + 1:sc + 2]
                    nc.vector.tensor_copy(t32[:, 0:h], raw[:, j, :])
                    nc.vector.tensor_scalar(t32b[:, 0:h], t32[:, 0:h], 15,
                                            None, op0=ALU.bitwise_and)
                    nc.vector.tensor_copy(tf[:, 0:h], t32b[:, 0:h])
                    nc.scalar.activation(dst[:, j, 0:h], tf[:, 0:h],
                                         AF.Identity, bias=bcol, scale=scol)
                    nc.vector.tensor_scalar(t32b[:, 0:h], t32[:, 0:h], 4,
                                            None, op0=ALU.logical_shift_right)
                    nc.vector.tensor_copy(tf[:, 0:h], t32b[:, 0:h])
                    nc.scalar.activation(dst[:, j, h:M], tf[:, 0:h],
                                         AF.Identity, bias=bcol, scale=scol)

        # ---------------- projections + attention ----------------
        # One PSUM budget for both phases (D 1 + lt 3x2 + a 1 = 8 banks).
        # Proj psum tiles ride the lt-slot rotation, emitted inside chunk 0's
        # kt loop right before the lt that consumes them, so attention starts
        # immediately and the proj copies drain on DVE behind the exp stream.
        with tc.tile_pool(name="d_ps", bufs=1, space="PSUM") as dps, \
             tc.tile_pool(name="att_lt", bufs=3, space="PSUM") as lps, \
             tc.tile_pool(name="acc_ps", bufs=1, space="PSUM") as cps, \
             tc.tile_pool(name="attsb", bufs=3) as attsb, \
             tc.tile_pool(name="wTp", bufs=34) as wTp, \
             tc.tile_pool(name="bcast", bufs=2) as bcp, \
             tc.tile_pool(name="outp", bufs=3) as outp:

            def emit_proj_block(nk):
                for ht in (4 * nk, 4 * nk + 1, 4 * nk + 2, 4 * nk + 3):
                    ps = lps.tile([128, C], f32, tag="lt_ps", name=f"vpps{ht}")
                    nc.tensor.matmul(ps, kvn8[:, :, ht * 128:(ht + 1) * 128], wv8,
                                     start=True, stop=True, perf_mode=DR)
                    nc.vector.tensor_copy(vpT_all[:, ht, :], ps)
                for ct in range(CT):
                    csl = slice(ct * 128, (ct + 1) * 128)
                    ps = lps.tile([128, 512], f32, tag="lt_ps",
                                  name=f"kpps{ct}_{nk}")
                    nc.tensor.matmul(ps, wk8[:, :, csl],
                                     kvn8[:, :, nk * 512:(nk + 1) * 512],
                                     start=True, stop=True, perf_mode=DR)
                    nc.vector.tensor_scalar_mul(
                        kp8[:, ct, nk * 512:(nk + 1) * 512], ps, 1.0 / WS)

            # single persistent [1, 512] denominator bank; chunks reuse it
            # (WAR on the rD read serializes only the chunk seam)
            Dall = dps.tile([1, 512], f32, tag="d_ps", name="Dall")
            pending = None  # (wTs, rDb, q0, N) of the previous chunk

            def drain_applies():
                wTs, rDb, q0, N = pending
                nr, r0 = N // W, q0 // W
                for ct in range(CT):
                    csl = slice(ct * 128, (ct + 1) * 128)
                    a_ps = cps.tile([128, nr, W], f32, tag="a_ps",
                                    name=f"aps{q0}_{ct}")
                    for ktp in range(KT // 2):
                        nc.tensor.matmul(
                            a_ps, vpT_all[:, 2 * ktp:2 * ktp + 2, csl], wTs[ktp],
                            start=(ktp == 0), stop=(ktp == KT // 2 - 1),
                            perf_mode=DR)
                    nc.vector.tensor_mul(a_pad8[:, ct, r0:r0 + nr, 1:W + 1],
                                         a_ps, rDb)

            def conv_block(nk):
                # conv rows 8nk..8nk+7; a_pad rows 8nk..8nk+9 are final.
                # Shares the a-bank psum tag and runs on DVE so the exp
                # stream on ACT is untouched.
                for ct in range(CT):
                    csl = slice(ct * 128, (ct + 1) * 128)
                    ps = cps.tile([128, 8, W], f32, tag="a_ps",
                                  name=f"cps{ct}_{nk}")
                    idx = 0
                    for dy in range(3):
                        for dx in range(3):
                            nc.tensor.matmul(
                                ps, wo8[(dy, dx)][:, :, csl],
                                a_pad8[:, :, 8 * nk + dy:8 * nk + dy + 8,
                                       dx:dx + W],
                                start=(idx == 0), stop=(idx == 8), perf_mode=DR)
                            idx += 1
                    osb = outp.tile([128, 512], f8, tag="cv_out",
                                    name=f"cvo{ct}_{nk}")
                    nc.vector.tensor_scalar_mul(
                        osb, ps.rearrange("p r w -> p (r w)"), OSC / (OS * AS))
                    nc.sync.dma_start(
                        out_d[ct * 128:(ct + 1) * 128,
                              nk * 512:(nk + 1) * 512],
                        osb)

            for ci, (q0, N) in enumerate(CHUNKS):
                nr = N // W
                qp8 = attsb.tile([128, 2, N], f8, tag="qp_sb", name=f"qp8_{ci}")
                for ct in range(CT):
                    csl = slice(ct * 128, (ct + 1) * 128)
                    ps = lps.tile([128, N], f32, tag="lt_ps", name=f"qpps{ci}_{ct}")
                    nc.tensor.matmul(ps, wq8[:, :, csl], qn8[:, :, q0:q0 + N],
                                     start=True, stop=True, perf_mode=DR)
                    nc.scalar.activation(qp8[:, ct, :], ps, AF.Identity,
                                         bias=bq_sb[ct], scale=1.0 / WS)
                Dp = Dall[:, 0:N]
                wTs = []
                for ktp in range(KT // 2):
                    if ci == 0 and ktp % 2 == 0:
                        emit_proj_block(ktp // 2)
                    wT8 = wTp.tile([128, 2, N], f8, tag="wT", name=f"wT{ci}_{ktp}")
                    lt2 = lps.tile([128, 2, N], f32, tag="lt_ps",
                                   name=f"lt{ci}_{ktp}")
                    for j in range(2):
                        kt = 2 * ktp + j
                        nc.tensor.matmul(lt2[:, j, :],
                                         kp8[:, :, kt * 128:(kt + 1) * 128],
                                         qp8, start=True, stop=True, perf_mode=DR)
                    if 1 <= ci <= 3 and ktp % 4 == 2:
                        # offload some exps to DVE (Schraudolph bitcast exp,
                        # +-3% -- noise floor is set by fp8 anyway)
                        ti = attsb.tile([128, 2, N], mybir.dt.int32, tag="ei32",
                                        name=f"ei{ci}_{ktp}")
                        nc.vector.tensor_scalar(
                            ti, lt2, EXP_A, EXP_B, op0=mybir.AluOpType.mult,
                            op1=mybir.AluOpType.add)
                        nc.vector.tensor_copy(wT8, ti.bitcast(f32))
                    else:
                        nc.scalar.activation(wT8, lt2, AF.Exp, scale=SC)
                    nc.tensor.matmul(Dp, ones8[:, :, 0:1], wT8, start=(ktp == 0),
                                     stop=(ktp == KT // 2 - 1), perf_mode=DR)
                    wTs.append(wT8)
                rD = attsb.tile([1, N], f32, tag="rD", name=f"rD{ci}")
                nc.vector.reciprocal(rD, Dp)
                nc.vector.tensor_mul(rD, rD, rowmask_sb[0:1, q0:q0 + N])
                rDb = bcp.tile([128, nr, W], f32, tag="rDb", name=f"rDb{ci}")
                nc.gpsimd.partition_broadcast(rDb, rD)
                # apply matmuls run one chunk behind the exp stream so the PE
                # burst never sits between this chunk's exps and the next's
                # logits in the PE queue; conv blocks trail one further chunk
                if pending is not None:
                    drain_applies()
                    if ci >= 2:
                        conv_block(ci - 2)
                pending = (wTs, rDb, q0, N)
            drain_applies()
            conv_block(3)

    nc.compile()
    return nc


def _make_runner(nc, n_cores=8):
    """Builds a cached jit of the bass program. Output zero-buffers are
    created inside the jit body (no separate device allocation dispatch);
    the kernel writes every output element so their content is never read."""
    import jax
    import jax.numpy as jnp
    import numpy as _np
    from jax.sharding import Mesh, PartitionSpec, NamedSharding
    from jax.experimental.shard_map import shard_map
    from concourse import mybir
    from concourse.bass2jax import (_bass_exec_p, install_neuronx_cc_hook,
                                    partition_id_tensor)

    install_neuronx_cc_hook()

    partition_name = nc.partition_id_tensor.name if nc.partition_id_tensor else None
    in_names, out_names, out_avals = [], [], []
    for alloc in nc.m.functions[0].allocations:
        if not isinstance(alloc, mybir.MemoryLocationSet):
            continue
        name = alloc.memorylocations[0].name
        if alloc.kind == "ExternalInput":
            if name != partition_name:
                in_names.append(name)
        elif alloc.kind == "ExternalOutput":
            shape = tuple(alloc.tensor_shape)
            np_dt = mybir.dt.np(alloc.dtype)
            out_names.append(name)
            out_avals.append(jax.core.ShapedArray(shape, np_dt))

    n_params = len(in_names)
    all_in_names = in_names + out_names
    if partition_name is not None:
        all_in_names.append(partition_name)

    def _body(*args):
        operands = list(args)
        if partition_name is not None:
            operands.append(partition_id_tensor())
        outs = _bass_exec_p.bind(
            *operands,
            out_avals=tuple(out_avals),
            in_names=tuple(all_in_names),
            out_names=tuple(out_names),
            lowering_input_output_aliases=(),
            sim_require_finite=True,
            sim_require_nnan=True,
            nc=nc,
        )
        return tuple(outs)

    devices = jax.devices()[:n_cores]
    mesh = Mesh(_np.asarray(devices), ("core",))
    n_outs = len(out_names)
    in_specs = (PartitionSpec("core"),) * (n_params + n_outs)
    out_specs = (PartitionSpec("core"),) * n_outs
    # The out buffers are passed as cached NON-donated zero inputs: the
    # kernel writes every output element, so their content is never read and
    # one committed device array can be reused across calls (no per-call
    # allocation dispatch, no transfer).
    sharded = jax.jit(
        shard_map(_body, mesh=mesh, in_specs=in_specs, out_specs=out_specs,
                  check_rep=False))
    shard = NamedSharding(mesh, PartitionSpec("core"))
    import jax.numpy as _jnp
    zero_devs = [
        _jnp.zeros((n_cores * a.shape[0], *a.shape[1:]), a.dtype, device=shard)
        for a in out_avals
    ]
    return sharded, shard, in_names, out_names, zero_devs


def _pack_static(wq, bq, wkv, bkv, wo, gn_w, gn_b, bo):
    """Device-static arrays (weight pack, rowmask) + host-side bias map."""
    wq = np.asarray(wq, np.float32)
    wkv = np.asarray(wkv, np.float32)
    wo = np.asarray(wo, np.float32)
    wk = wkv[0::2]
    wv = wkv[1::2]
    bv = np.asarray(bkv, np.float32)[1::2]

    woT = wo.transpose(1, 2, 3, 0).reshape(C, 9 * C)  # [ci, (dy dx co)]
    wpack = np.concatenate([wq.T * WS, wk.T * WS, wv.T * WS, woT * OS], axis=1)
    wpack = np.clip(wpack, -240.0, 240.0)
    wpack8 = wpack.astype(F8).reshape(2, 128, 12 * C).transpose(1, 0, 2)
    # replicate per core and flatten the core axis into the shard axis
    wpack8 = np.ascontiguousarray(
        np.broadcast_to(wpack8, (8, 128, 2, 12 * C))).reshape(8 * 128, 2, 12 * C)

    # rowmask: AS*SC softmax scaling, zeroed on the out-of-image halo row
    rowmask = np.empty((8, NQ), np.float32)
    for core in range(8):
        m = np.full((NROWS, W), AS * SC, np.float32)
        if core % 2 == 0:
            m[0] = 0.0
        else:
            m[NROWS - 1] = 0.0
        rowmask[core] = m.reshape(NQ)

    # bv enters the output linearly: a = a_nobias + bv[c]  =>
    # out += conv3x3(bv_map) with SAME zero padding; bo is added here too.
    # (bk is a softmax no-op and is dropped.)
    tap = np.einsum("oikl,i->okl", wo, bv)  # [C_out, 3, 3]
    bias_map = np.zeros((C, H, W), np.float32)
    for dy in range(3):
        for dx in range(3):
            y0, y1 = max(0, 1 - dy), min(H, H + 1 - dy)
            x0, x1 = max(0, 1 - dx), min(W, W + 1 - dx)
            bias_map[:, y0:y1, x0:x1] += tap[:, dy, dx][:, None, None]
    bias_map += np.asarray(bo, np.float32)[:, None, None]
    return wpack8, rowmask, bias_map


def kernel(q, kv, gn_w, gn_b, wq, bq, wkv, bkv, wo, bo):
    import jax

    if "run" not in _CACHE:
        nc = _build()
        _CACHE["run"] = _make_runner(nc)
    sharded, shard, in_names, out_names, zero_devs = _CACHE["run"]

    q = np.asarray(q, np.float32).reshape(B, C, HW)
    kv = np.asarray(kv, np.float32).reshape(B, C, HW)

    # ---- static (weight) arrays: cache committed device buffers ----
    wlist = (wq, bq, wkv, bkv, wo, bo, gn_w, gn_b)
    st = _CACHE.get("static")
    if st is None or not all(
            np.array_equal(np.asarray(a, np.float32), b)
            for a, b in zip(wlist, st["wlist"])):
        wpack8, rowmask, bias_map = _pack_static(
            wq, bq, wkv, bkv, wo, gn_w, gn_b, bo)
        st = {
            "wlist": [np.array(np.asarray(a, np.float32)) for a in wlist],
            "bias_map": bias_map,
            "wpack_dev": jax.device_put(np.ascontiguousarray(wpack8), shard),
            "rowmask_dev": jax.device_put(rowmask, shard),
        }
        _CACHE["static"] = st

    # ---- dynamic prep: GN stats on host, int4 raw quantization ----------
    gw = np.asarray(gn_w, np.float32)
    gb = np.asarray(gn_b, np.float32)
    bqv = np.asarray(bq, np.float32)

    def gn_stats(x):
        xg = x.reshape(B, 32, 8 * HW)
        m = xg.mean(axis=2)
        v = xg.var(axis=2)
        rstd = 1.0 / np.sqrt(v + EPS)           # [B, 32]
        scol = gw[None, :] * np.repeat(rstd, 8, axis=1)    # [B, C]
        bcol = gb[None, :] - np.repeat(m, 8, axis=1) * scol
        # per-sample int4 grid: conservative |x| bound from the group stats
        s = np.maximum((np.abs(m) + 5.0 * np.sqrt(v)).max(axis=1) / 7.5, 1e-6)
        return scol, bcol, s.astype(np.float32)

    def quant4(x, s):
        y = x * (1.0 / s)[:, None, None]
        y += 8.0
        np.clip(y, 0.0, 15.99, out=y)
        return y.astype(np.uint8)

    cols = np.empty((B, C, 5), np.float32)

    # kv first so its transfer overlaps the q-side host work
    scol, bcol, s_kv = gn_stats(kv)
    cols[:, :, 2] = scol * s_kv[:, None]
    cols[:, :, 3] = bcol - 7.5 * s_kv[:, None] * scol
    nkv = quant4(kv, s_kv)                      # [B, C, HW]
    pk = nkv[:, :, :HW // 2] | (nkv[:, :, HW // 2:] << 4)
    pkT = pk.reshape(B, 2, 128, HW // 2).transpose(0, 2, 1, 3)
    Ukv = np.empty((8, 128, 2, HW // 2), np.uint8)
    Ukv[0::2] = pkT
    Ukv[1::2] = pkT
    dKV = jax.device_put(Ukv.reshape(8 * 128, 2, HW // 2), shard)

    scol, bcol, s_q = gn_stats(q)
    cols[:, :, 0] = scol * s_q[:, None]
    cols[:, :, 1] = bcol - 7.5 * s_q[:, None] * scol
    cols[:, :, 4] = bqv[None, :]
    nq = quant4(q, s_q).reshape(B, 2, 128, H, W).transpose(0, 2, 1, 3, 4)
    # q34 flat [NROWS, W] split at row 17 for the nibble halves; the halo
    # rows (0 for top cores, 33 for bottom) are arbitrary -- rowmask zeroes
    # their attention output before the conv reads them.
    lo = np.zeros((B, 128, 2, NQ // 2), np.uint8)
    hi = np.empty((B, 128, 2, NQ // 2), np.uint8)
    lo[:, :, :, W:] = nq[:, :, :, 0:16].reshape(B, 128, 2, 16 * W)
    hi[:, :, :, :] = nq[:, :, :, 16:33].reshape(B, 128, 2, 17 * W)
    Uq = np.empty((8, 128, 2, NQ // 2), np.uint8)
    Uq[0::2] = lo | (hi << 4)
    lo2 = nq[:, :, :, 31:48].reshape(B, 128, 2, 17 * W)
    hi[:, :, :, :16 * W] = nq[:, :, :, 48:64].reshape(B, 128, 2, 16 * W)
    hi[:, :, :, 16 * W:] = 0
    Uq[1::2] = lo2 | (hi << 4)
    dQ = jax.device_put(Uq.reshape(8 * 128, 2, NQ // 2), shard)

    cols_pc = cols.reshape(B, 2, 128, 5).transpose(0, 2, 1, 3)   # [B,128,2,5]
    cols_up = np.repeat(cols_pc, 2, axis=0)                      # [8,128,2,5]
    arrs = {
        "kv4": dKV,
        "q4": dQ,
        "cols": jax.device_put(
            np.ascontiguousarray(cols_up).reshape(8 * 128, 2, 5), shard),
        "wpack": st["wpack_dev"],
        "rowmask": st["rowmask_dev"],
    }
    fut = sharded(*[arrs[n] for n in in_names], *zero_devs)

    # host residual overlaps the device round trip
    base = q.reshape(B, C, H, W) + st["bias_map"][None]
    lut = _CACHE.get("lut")
    if lut is None:
        lut = (np.arange(256, dtype=np.uint8).view(F8).astype(np.float32)
               * (1.0 / OSC))
        _CACHE["lut"] = lut
    d = lut[np.asarray(fut[0]).view(np.uint8)]   # [8*C, NOUT] f32
    d = d.reshape(B, 2, C, 32, W).transpose(0, 2, 1, 3, 4).reshape(B, C, H, W)
    base += d
    return base
